# revision 37
# baseline (speedup 1.0000x reference)
"""Trainium2 Bass kernel for nn_Attention_45724221833663 (sparse_attention).

Strategy: data-parallel over batch B=8 across the 8 NeuronCores (one batch
element per core). All matmuls run in bf16 with fp32 PSUM accumulation.

Per-core dataflow (all layouts chosen to avoid on-chip transposes of large
activations; weights and x are transposed on the host while sharding, and
ln_g/ln_b are folded into Wp/bp on the host):
  xcatT  [c=1024, kvp=1152]  (= concat(x_text, x).T, zero-padded 1101->1152)
  vw     [kvp, h, 65] = (xcatT.T @ WvT) interleaved per head + ones column
  qT     [o, n]    = WqT.T @ xT          (o = head-major channel)
  kT     [o, kvp]  = WkT.T @ xcatT
  per head h:
    scoresT[kv, n] = kT_h contracted with qT_h over d=64
    E = exp(scoresT / 8)     (ScalarE, psum -> sbuf bf16); row kv=0 and the
                             pad rows are zeroed
    avp[n,0:65] = sum_kv E[kv,n-tile] * vw[kv, h, :]   (col 64 = S[n])
    attn[n, h*64:+64] = avp[:, :64] * (1/S) + tanh(g_h) * v_h[kv=0]
  LayerNorm over channels (rows of attn, bf16 input like the reference's
  bf16 cast; ln_g/ln_b pre-folded), then out = LN @ Wp'.T + bp' with the
  bias added as a rank-1 matmul and the result DMA'd PSUM -> DRAM.
"""

import os
import numpy as np
import ml_dtypes

import concourse.bacc as bacc
import concourse.tile as tile
from concourse import mybir
from concourse.masks import make_identity
from concourse.bass_utils import run_bass_kernel_spmd

F32 = mybir.dt.float32
BF16 = mybir.dt.bfloat16
F8 = mybir.dt.float8e4
DR = mybir.MatmulPerfMode.DoubleRow
AF = mybir.ActivationFunctionType
OP = mybir.AluOpType
VSCALE = 16.0  # fp8 v/ones scaling; cancels in avp * (1/S-column)

B, N, P, DIM, H = 8, 1024, 77, 1024, 16
HD = DIM // H          # 64
KV = P + N             # 1101
KT = 9                 # kv tiles of 128
KVP = KT * 128         # 1152 padded
NT = N // 128          # 8 n tiles
CC = DIM // 128        # 8 contraction chunks
OT = DIM // 128        # 8 output-channel tiles
LN_EPS = 1e-5

LAST_EXEC_NS = None
_CACHE = {}


def _emit(tc):
    nc = tc.nc

    xcatT_d = nc.dram_tensor("xcatT", [DIM, KVP], BF16, kind="ExternalInput").ap()
    wq_d = nc.dram_tensor("wqT", [DIM, DIM], BF16, kind="ExternalInput").ap()
    wk_d = nc.dram_tensor("wkT", [DIM, DIM], BF16, kind="ExternalInput").ap()
    wv_d = nc.dram_tensor("wvT", [DIM, DIM], BF16, kind="ExternalInput").ap()
    wp_d = nc.dram_tensor("wpT", [DIM, DIM], BF16, kind="ExternalInput").ap()
    tanhg_d = nc.dram_tensor("tanhg", [1, H], F32, kind="ExternalInput").ap()
    bp_d = nc.dram_tensor("bp_bf", [1, DIM], BF16, kind="ExternalInput").ap()
    out_d = nc.dram_tensor("out", [N, DIM], F32, kind="ExternalOutput").ap()

    xcat_re = xcatT_d.rearrange("(j p) f -> p j f", p=128)
    wq_re = wq_d.rearrange("(j p) o -> p j o", p=128)
    wk_re = wk_d.rearrange("(j p) o -> p j o", p=128)
    wv_re = wv_d.rearrange("(j p) o -> p j o", p=128)
    wp_re = wp_d.rearrange("(j p) o -> p j o", p=128)

    from contextlib import ExitStack

    with ExitStack() as top:
        consts = top.enter_context(tc.tile_pool(name="consts", bufs=1))
        acts = top.enter_context(tc.tile_pool(name="acts", bufs=1))
        ph1 = top.enter_context(tc.tile_pool(name="ph1", bufs=1))
        wstream = top.enter_context(tc.tile_pool(name="wstream", bufs=3))
        qkp = top.enter_context(tc.tile_pool(name="qkp", bufs=7))
        epool = top.enter_context(tc.tile_pool(name="epool", bufs=9))
        tpool = top.enter_context(tc.tile_pool(name="tmp", bufs=4))
        ltp = top.enter_context(tc.tile_pool(name="ltp", bufs=6))
        opool = top.enter_context(tc.tile_pool(name="outp", bufs=3))
        ps_proj = top.enter_context(tc.tile_pool(name="ps_proj", bufs=2, space="PSUM"))
        ps_scores = top.enter_context(
            tc.tile_pool(name="ps_scores", bufs=2, space="PSUM"))
        ps_av = top.enter_context(tc.tile_pool(name="ps_av", bufs=2, space="PSUM"))

        # ---- constants ----
        tanhg_sb = consts.tile([128, H], F32, tag="tanhg")
        nc.sync.dma_start(out=tanhg_sb, in_=tanhg_d.to_broadcast([128, H]))
        bp_sb = consts.tile([1, DIM], BF16, tag="bp")
        nc.sync.dma_start(out=bp_sb, in_=bp_d)
        eps_t = consts.tile([128, 1], F32, tag="eps")
        nc.vector.memset(eps_t, LN_EPS)
        zcol = consts.tile([128, 1], F32, tag="zcol")
        nc.vector.memset(zcol, 0.0)
        # bias row broadcast across partitions: added via DVE stt at the
        # psum->sbuf drain instead of rank-1 matmuls on the PE
        bp128 = consts.tile([128, DIM], BF16, tag="bp128")
        nc.gpsimd.partition_broadcast(bp128, bp_sb)
        ident = consts.tile([128, 128], BF16, tag="ident")
        make_identity(nc, ident)

        # ---- persistent activations ----
        # vw holds VSCALE*v (+ VSCALE ones column) in fp8 for DoubleRow AV
        vw_sb = acts.tile([128, KT, H, HD + 1], F8, tag="vw")  # [kv-part, kv-tile, h, d+1]
        attn_sb = acts.tile([128, NT, H, HD], BF16, tag="attn")  # [n-part, n-tile, h, d]
        v0row = consts.tile([1, DIM], BF16, tag="v0row")  # v[kv=0] kept bf16-precise

        # input loads, c-chunk granular; only xcatT is loaded up front --
        # wv/wp loads are emitted later, in consumption order
        xcatT_sb = ph1.tile([128, CC, KVP], BF16, tag="xcatT")
        # wv and wp share one slot: wv dies after the v projection, wp is
        # only needed from the output projection onwards
        wv_sb = ph1.tile([128, CC, DIM], BF16, tag="wvwp")
        w0q = wstream.tile([128, CC, 128], BF16, tag="w")
        nc.sync.dma_start(out=w0q, in_=wq_re[:, :, 0:128])
        w0k = wstream.tile([128, CC, 128], BF16, tag="w")
        nc.sync.dma_start(out=w0k, in_=wk_re[:, :, 0:128])
        for cc in range(CC):
            dmae = nc.sync if cc % 3 != 2 else nc.gpsimd
            dmae.dma_start(out=xcatT_sb[:, cc, :], in_=xcat_re[:, cc, :])

        # ---- q/k projections interleaved with their dependent head pairs,
        # so ScalarE (exp) fills while PE still runs projections ----
        last_rows = KV - (KT - 1) * 128  # 77
        ksplits = [(0, 512), (512, 512), (1024, 128)]

        def emit_vproj_chunk(kvt, half):
            # one (kv-tile, head-half) of the v projection, natural [kv, o]
            ps = ps_proj.tile([128, 512], F32, tag="ps")
            for cc in range(CC):
                nc.tensor.matmul(
                    ps,
                    xcatT_sb[:, cc, kvt * 128:(kvt + 1) * 128],
                    wv_sb[:, cc, half * 512:(half + 1) * 512],
                    start=(cc == 0),
                    stop=(cc == CC - 1),
                )
            if kvt == 0:
                # bf16-precise first-key row for the gate term
                nc.vector.tensor_copy(
                    v0row[0:1, half * 512:(half + 1) * 512], ps[0:1, :])
            nc.vector.tensor_scalar_mul(
                vw_sb[:, kvt, half * 8:(half + 1) * 8, 0:HD],
                ps.rearrange("p (h d) -> p h d", d=HD),
                VSCALE,
            )

        # PE work other than the score matmuls is drip-fed between score
        # tiles via a budget-paced filler queue, so the PE never runs a
        # long burst that starves ScalarE of score psums. Entries are
        # (group, est_us, thunk); pop_fill spends ~a slot's budget, and
        # drain_group forces everything up to a group out (emission-order
        # dependencies: qk(p) must be emitted before scores(p) reads it).
        fill_q = []

        def pop_fill(budget=1.3):
            spent = 0.0
            while fill_q and spent < budget:
                _, c, t = fill_q.pop(0)
                t()
                spent += c

        def drain_group(g):
            # groups touch disjoint tiles, so popping out of FIFO order is
            # safe; only the named group is forced out
            rest = []
            for e in fill_q:
                if e[0] == g:
                    e[2]()
                else:
                    rest.append(e)
            fill_q[:] = rest

        def qk_q_half(qt, wtq, half):
            ps = ps_proj.tile([128, 512], F32, tag="ps")
            for cc in range(CC):
                nc.tensor.matmul(
                    ps,
                    wtq[:, cc, :],
                    xcatT_sb[:, cc, P + half * 512: P + (half + 1) * 512],
                    start=(cc == 0),
                    stop=(cc == CC - 1),
                )
            nc.vector.tensor_copy(qt[:, half * 512:(half + 1) * 512], ps)

        def qk_k_chunk(kte, kto, wtk, off, width):
            # k is split per head into K=128 zero-padded tiles (kte rows
            # 64:128 and kto rows 0:64 stay zero) so the score matmuls
            # contract a full 128 partitions (keeps fast-weight-load on)
            ps = ps_proj.tile([128, 512], F32, tag="ps")
            for cc in range(CC):
                nc.tensor.matmul(
                    ps[:, :width],
                    wtk[:, cc, :],
                    xcatT_sb[:, cc, off:off + width],
                    start=(cc == 0),
                    stop=(cc == CC - 1),
                )
            nc.vector.tensor_copy(kte[0:64, off:off + width], ps[0:64, :width])
            nc.vector.tensor_copy(kto[64:128, off:off + width], ps[64:128, :width])

        def push_qk(ot, wtq=None, wtk=None, direct=False):
            qt = qkp.tile([128, N], BF16, tag="qt")
            kte = qkp.tile([128, KVP], BF16, tag="kte")
            kto = qkp.tile([128, KVP], BF16, tag="kto")
            nc.gpsimd.memset(kte[64:128, :], 0.0)
            nc.gpsimd.memset(kto[0:64, :], 0.0)
            if wtq is None:
                wtq = wstream.tile([128, CC, 128], BF16, tag="w")
                nc.sync.dma_start(out=wtq, in_=wq_re[:, :, ot * 128:(ot + 1) * 128])
            if wtk is None:
                wtk = wstream.tile([128, CC, 128], BF16, tag="w")
                nc.sync.dma_start(out=wtk, in_=wk_re[:, :, ot * 128:(ot + 1) * 128])
            grp = f"qk{ot}"
            for half in range(2):
                fill_q.append((grp, 1.73,
                               lambda half=half: qk_q_half(qt, wtq, half)))
            for off, width in ksplits:
                fill_q.append((grp, 1.73 * width / 512,
                               lambda off=off, width=width:
                               qk_k_chunk(kte, kto, wtk, off, width)))
            if direct:
                drain_group(grp)
            return qt, (kte, kto)

        def emit_scores_pair(qt, kt):
            # Scores for the even/odd head pair, row-tiled on the PE
            # (K=64 each, partitions 0-63 and 64-127 run concurrently).
            ee = epool.tile([128, KT, N], F8, tag="e")
            eo = epool.tile([128, KT, N], F8, tag="e")
            nc.gpsimd.memset(ee[:, KT - 1, :], 0.0)
            nc.gpsimd.memset(eo[:, KT - 1, :], 0.0)
            kte, kto = kt
            for kvt in range(KT):
                pse = ps_scores.tile([128, N], F32, tag="pss")
                pso = ps_scores.tile([128, N], F32, tag="pss")
                for half in range(2):
                    nc.tensor.matmul(
                        pse[:, half * 512:(half + 1) * 512],
                        kte[:, kvt * 128:(kvt + 1) * 128],
                        qt[:, half * 512:(half + 1) * 512],
                        start=True, stop=True,
                    )
                    nc.tensor.matmul(
                        pso[:, half * 512:(half + 1) * 512],
                        kto[:, kvt * 128:(kvt + 1) * 128],
                        qt[:, half * 512:(half + 1) * 512],
                        start=True, stop=True,
                    )
                rows = last_rows if kvt == KT - 1 else 128
                nc.scalar.activation(
                    ee[:rows, kvt, :], pse[:rows], AF.Exp, bias=0.0, scale=0.125)
                nc.scalar.activation(
                    eo[:rows, kvt, :], pso[:rows], AF.Exp, bias=0.0, scale=0.125)
                pop_fill()
            # first key column is gated separately
            nc.gpsimd.memset(ee[0:1, 0, :], 0.0)
            nc.gpsimd.memset(eo[0:1, 0, :], 0.0)
            return ee, eo

        # fp8 AV: plain (non-DoubleRow) keeps fast-weight-load enabled,
        # which wins for this small free dim (65) where LDWEIGHTS dominates
        AV_DR = bool(int(os.environ.get("BASS_AV_DR", "0")))

        def av_chunk(h, e, nts, gv0s, alt):
            for nt in nts:
                if alt and nt % 2 == 1:
                    avp = ps_proj.tile([128, HD + 1], F32, tag="ps")
                else:
                    avp = ps_av.tile([128, HD + 1], F32, tag="avp")
                if AV_DR:
                    # DoubleRow: contract two kv-tiles per matmul
                    for j in range(KT // 2):
                        nc.tensor.matmul(
                            avp,
                            e[:, 2 * j:2 * j + 2, nt * 128:(nt + 1) * 128],
                            vw_sb[:, 2 * j:2 * j + 2, h, :],
                            start=(j == 0),
                            stop=False,
                            perf_mode=DR,
                        )
                else:
                    for j in range(KT - 1):
                        nc.tensor.matmul(
                            avp,
                            e[:, j, nt * 128:(nt + 1) * 128],
                            vw_sb[:, j, h, :],
                            start=(j == 0),
                            stop=False,
                        )
                nc.tensor.matmul(
                    avp,
                    e[:, KT - 1, nt * 128:(nt + 1) * 128],
                    vw_sb[:, KT - 1, h, :],
                    start=False,
                    stop=True,
                )
                rs = tpool.tile([128, 1], F32, tag="rs")
                nc.vector.reciprocal(rs, avp[:, HD:HD + 1])
                nc.vector.scalar_tensor_tensor(
                    out=attn_sb[:, nt, h, :],
                    in0=avp[:, 0:HD],
                    scalar=rs,
                    in1=gv0s,
                    op0=OP.mult,
                    op1=OP.add,
                )

        def push_av(pair, es, alt=False, nt_major=False):
            """Queue the AV + fixup work for both heads of a pair as small
            fillers. nt_major orders chunks so attn n-tiles complete in
            order (lets the LN/out-proj pipeline start during the drain)."""
            gv = []
            for i, h in enumerate((2 * pair, 2 * pair + 1)):
                gv0 = tpool.tile([128, HD], BF16, tag="gv0")
                nc.gpsimd.partition_broadcast(
                    gv0, v0row[0:1, h * HD:(h + 1) * HD])
                gv0s = tpool.tile([128, HD], F32, tag="gv0s")
                nc.vector.tensor_scalar_mul(gv0s, gv0, tanhg_sb[:, h:h + 1])
                gv.append(gv0s)
            grp = f"av{pair}"
            for nt0 in range(0, NT, 2):
                for i, h in enumerate((2 * pair, 2 * pair + 1)):
                    nts = [nt0, nt0 + 1]
                    fill_q.append(
                        (grp, 0.6,
                         lambda h=h, e=es[i], nts=tuple(nts), g=gv[i]:
                         av_chunk(h, e, nts, g, alt)))

        # ---- software pipeline over head pairs: scores+exp stream on
        # PE+ScalarE while qk projections, the v projection, and the AV
        # stages of earlier pairs fill the PE between score tiles ----
        pend = []
        qt, kt = push_qk(0, w0q, w0k, direct=True)
        for cc in range(CC):
            nc.sync.dma_start(out=wv_sb[:, cc, :], in_=wv_re[:, cc, :])
        # ones column for the row-sum S (E rows for kv=0/pad are zeroed);
        # disjoint from the v-projection's columns, so set it up front
        nc.gpsimd.memset(vw_sb[:, :, :, HD:HD + 1], VSCALE)
        nxt = push_qk(1)
        for kvt in range(KT):
            for half in range(2):
                fill_q.append(("vproj", 1.73,
                               lambda kvt=kvt, half=half:
                               emit_vproj_chunk(kvt, half)))
        pend.append(emit_scores_pair(qt, kt))
        for p in range(1, OT):
            qt, kt = nxt
            if p + 1 < OT:
                nxt = push_qk(p + 1)
            if p >= 3:
                # pair 4 drains after the last scores, where ps_proj is free
                push_av(p - 3, pend[p - 3], alt=(p - 3 >= 4))
            drain_group(f"qk{p}")
            pend.append(emit_scores_pair(qt, kt))
        wp_sb = ph1.tile([128, CC, DIM], BF16, tag="wvwp")
        for cc in range(CC):
            nc.sync.dma_start(out=wp_sb[:, cc, :], in_=wp_re[:, cc, :])
        # pairs 5..7 drain after the last scores; keep nt-major order on the
        # final pair so the LN/out-proj pipeline can start as tiles finish
        push_av(5, pend[5], alt=True)
        push_av(6, pend[6], alt=True)
        push_av(7, pend[7], alt=True, nt_major=True)
        while fill_q:
            _, _, t = fill_q.pop(0)
            t()

        # ---- LayerNorm + output projection per n-tile; LN emitted one
        # n-tile ahead so the LN chain of nt+1 overlaps the projection of nt ----
        def emit_ln(nt):
            xa = attn_sb[:, nt].rearrange("p h d -> p (h d)")
            xs = xa.rearrange("p (s f) -> p s f", f=512)
            stats = tpool.tile([128, 2, 6], F32, tag="stats")
            for s in range(2):
                nc.vector.bn_stats(stats[:, s, :], xs[:, s, :])
            mv = tpool.tile([128, 2], F32, tag="mv")
            nc.vector.bn_aggr(mv, stats)
            rstd = tpool.tile([128, 1], F32, tag="rstd")
            nc.scalar.activation(rstd, mv[:, 1:2], AF.Sqrt, bias=eps_t, scale=1.0)
            nc.vector.reciprocal(rstd, rstd)
            # ln_g/ln_b are folded into Wp/bp host-side: L = (x - mu) * rstd
            L_t = qkp.tile([128, DIM], BF16, tag="qt")
            nc.vector.tensor_scalar(
                out=L_t, in0=xa, scalar1=mv[:, 0:1], scalar2=rstd,
                op0=OP.subtract, op1=OP.mult,
            )
            return L_t

        L_t = emit_ln(0)
        for nt in range(NT):
            L_next = emit_ln(nt + 1) if nt + 1 < NT else None

            # transpose LN rows then project: out[n, o] = L @ Wp'.T + bp'
            pp0 = ps_proj.tile([128, 512], F32, tag="ps")
            pp1 = ps_proj.tile([128, 512], F32, tag="ps")
            for cc in range(CC):
                pstp, pstt = (ps_scores, "pss") if cc % 2 == 0 else (ps_av, "avp")
                pst = pstp.tile([128, 128], BF16, tag=pstt)
                nc.tensor.transpose(
                    pst, L_t[:, cc * 128:(cc + 1) * 128], ident
                )
                ltc = ltp.tile([128, 128], BF16, tag="ltc")
                nc.vector.tensor_copy(ltc, pst)
                nc.tensor.matmul(
                    pp0, ltc, wp_sb[:, cc, 0:512],
                    start=(cc == 0), stop=(cc == CC - 1),
                )
                nc.tensor.matmul(
                    pp1, ltc, wp_sb[:, cc, 512:1024],
                    start=(cc == 0), stop=(cc == CC - 1),
                )
            # psum -> sbuf drain on ScalarE (PSUM is not a legal DMA source);
            # the bias row is added by the otherwise-idle GpSimd engine so
            # neither the PE nor the tail-pacing DVE pays for it
            for half, pp in ((0, pp0), (1, pp1)):
                ot = opool.tile([128, 512], F32, tag="ot")
                nc.scalar.copy(out=ot, in_=pp)
                nc.gpsimd.tensor_tensor(
                    ot, ot, bp128[:, half * 512:(half + 1) * 512], OP.add)
                nc.sync.dma_start(
                    out=out_d[nt * 128:(nt + 1) * 128,
                              half * 512:(half + 1) * 512],
                    in_=ot)
            L_t = L_next


def build_program():
    if "nc" in _CACHE:
        return _CACHE["nc"]
    nc = bacc.Bacc("TRN2", target_bir_lowering=False, debug=False, num_devices=8)
    with tile.TileContext(nc) as tc:
        _emit(tc)
    nc.compile()
    _CACHE["nc"] = nc
    return nc


def prep_inputs(x, x_text, Wq, Wk, Wv, gate, ln_g, ln_b, Wp, bp):
    """Host-side sharding/layout prep. Returns the 8 per-core input maps."""
    bf = ml_dtypes.bfloat16
    x = np.asarray(x, np.float32)
    x_text = np.asarray(x_text, np.float32)
    xcat = np.concatenate([x_text, x], axis=1)          # [B, KV, DIM]
    xcatT = np.zeros((B, DIM, KVP), np.float32)
    xcatT[:, :, :KV] = xcat.transpose(0, 2, 1)
    xcatT = xcatT.astype(bf)
    wqT = np.ascontiguousarray(np.asarray(Wq, np.float32).T).astype(bf)
    wkT = np.ascontiguousarray(np.asarray(Wk, np.float32).T).astype(bf)
    wvT = np.ascontiguousarray(np.asarray(Wv, np.float32).T).astype(bf)
    # fold LayerNorm affine into the output projection:
    #   (L*g + b) @ Wp.T + bp == L @ (Wp*g).T + (bp + Wp @ b)
    Wp = np.asarray(Wp, np.float32)
    g = np.asarray(ln_g, np.float32).reshape(DIM)
    bvec = np.asarray(ln_b, np.float32).reshape(DIM)
    Wpf = Wp * g[None, :]
    bpf = np.asarray(bp, np.float32).reshape(DIM) + Wp @ bvec
    wpT = np.ascontiguousarray(Wpf.T).astype(bf)
    tanhg = np.tanh(np.asarray(gate, np.float32)).reshape(1, H).astype(np.float32)
    bp_bf = bpf.reshape(1, DIM).astype(bf)
    in_maps = []
    for b in range(B):
        in_maps.append({
            "xcatT": np.ascontiguousarray(xcatT[b]),
            "wqT": wqT, "wkT": wkT, "wvT": wvT, "wpT": wpT,
            "tanhg": tanhg, "bp_bf": bp_bf,
        })
    return in_maps


def kernel(**inputs):
    global LAST_EXEC_NS
    nc = build_program()
    in_maps = prep_inputs(**inputs)
    trace = bool(int(os.environ.get("BASS_TRACE_RUN", "0")))
    res = run_bass_kernel_spmd(
        nc, in_maps, core_ids=list(range(8)), trace=trace,
    )
    LAST_EXEC_NS = res.exec_time_ns
    out = np.stack([r["out"] for r in res.results], axis=0)
    return out.astype(np.float32)



# revision 41
# speedup vs baseline: 1.0687x; 1.0687x over previous
"""Trainium2 Bass kernel for nn_Attention_45724221833663 (sparse_attention).

Strategy: data-parallel over batch B=8 across the 8 NeuronCores (one batch
element per core). All matmuls run in bf16 with fp32 PSUM accumulation.

Per-core dataflow (all layouts chosen to avoid on-chip transposes of large
activations; weights and x are transposed on the host while sharding, and
ln_g/ln_b are folded into Wp/bp on the host):
  xcatT  [c=1024, kvp=1152]  (= concat(x_text, x).T, zero-padded 1101->1152)
  vw     [kvp, h, 65] = (xcatT.T @ WvT) interleaved per head + ones column
  qT     [o, n]    = WqT.T @ xT          (o = head-major channel)
  kT     [o, kvp]  = WkT.T @ xcatT
  per head h:
    scoresT[kv, n] = kT_h contracted with qT_h over d=64
    E = exp(scoresT / 8)     (ScalarE, psum -> sbuf bf16); row kv=0 and the
                             pad rows are zeroed
    avp[n,0:65] = sum_kv E[kv,n-tile] * vw[kv, h, :]   (col 64 = S[n])
    attn[n, h*64:+64] = avp[:, :64] * (1/S) + tanh(g_h) * v_h[kv=0]
  LayerNorm over channels (rows of attn, bf16 input like the reference's
  bf16 cast; ln_g/ln_b pre-folded), then out = LN @ Wp'.T + bp' with the
  bias added as a rank-1 matmul and the result DMA'd PSUM -> DRAM.
"""

import os
import numpy as np
import ml_dtypes

import concourse.bacc as bacc
import concourse.tile as tile
from concourse import mybir
from concourse.masks import make_identity
from concourse.bass_utils import run_bass_kernel_spmd

F32 = mybir.dt.float32
BF16 = mybir.dt.bfloat16
F8 = mybir.dt.float8e4
DR = mybir.MatmulPerfMode.DoubleRow
AF = mybir.ActivationFunctionType
OP = mybir.AluOpType
VSCALE = 16.0  # fp8 v/ones scaling; cancels in avp * (1/S-column)

B, N, P, DIM, H = 8, 1024, 77, 1024, 16
HD = DIM // H          # 64
KV = P + N             # 1101
KT = 9                 # kv tiles of 128
KVP = KT * 128         # 1152 padded
NT = N // 128          # 8 n tiles
CC = DIM // 128        # 8 contraction chunks
OT = DIM // 128        # 8 output-channel tiles
LN_EPS = 1e-5

LAST_EXEC_NS = None
_CACHE = {}


def _emit(tc):
    nc = tc.nc

    xcatT_d = nc.dram_tensor("xcatT", [DIM, KVP], BF16, kind="ExternalInput").ap()
    wq_d = nc.dram_tensor("wqT", [DIM, DIM], BF16, kind="ExternalInput").ap()
    wk_d = nc.dram_tensor("wkT", [DIM, DIM], BF16, kind="ExternalInput").ap()
    wv_d = nc.dram_tensor("wvT", [DIM, DIM], BF16, kind="ExternalInput").ap()
    wp_d = nc.dram_tensor("wpT", [DIM, DIM], BF16, kind="ExternalInput").ap()
    tanhg_d = nc.dram_tensor("tanhg", [1, H], F32, kind="ExternalInput").ap()
    bp_d = nc.dram_tensor("bp_bf", [1, DIM], BF16, kind="ExternalInput").ap()
    out_d = nc.dram_tensor("out", [N, DIM], F32, kind="ExternalOutput").ap()

    xcat_re = xcatT_d.rearrange("(j p) f -> p j f", p=128)
    wq_re = wq_d.rearrange("(j p) o -> p j o", p=128)
    wk_re = wk_d.rearrange("(j p) o -> p j o", p=128)
    wv_re = wv_d.rearrange("(j p) o -> p j o", p=128)
    wp_re = wp_d.rearrange("(j p) o -> p j o", p=128)

    from contextlib import ExitStack

    with ExitStack() as top:
        consts = top.enter_context(tc.tile_pool(name="consts", bufs=1))
        acts = top.enter_context(tc.tile_pool(name="acts", bufs=1))
        ph1 = top.enter_context(tc.tile_pool(name="ph1", bufs=1))
        wstream = top.enter_context(tc.tile_pool(name="wstream", bufs=3))
        qkp = top.enter_context(tc.tile_pool(name="qkp", bufs=7))
        epool = top.enter_context(tc.tile_pool(name="epool", bufs=9))
        tpool = top.enter_context(tc.tile_pool(name="tmp", bufs=4))
        ltp = top.enter_context(tc.tile_pool(name="ltp", bufs=6))
        opool = top.enter_context(tc.tile_pool(name="outp", bufs=3))
        ps_proj = top.enter_context(tc.tile_pool(name="ps_proj", bufs=2, space="PSUM"))
        ps_scores = top.enter_context(
            tc.tile_pool(name="ps_scores", bufs=2, space="PSUM"))
        ps_av = top.enter_context(tc.tile_pool(name="ps_av", bufs=2, space="PSUM"))

        # ---- constants ----
        tanhg_sb = consts.tile([128, H], F32, tag="tanhg")
        nc.sync.dma_start(out=tanhg_sb, in_=tanhg_d.to_broadcast([128, H]))
        bp_sb = consts.tile([1, DIM], BF16, tag="bp")
        nc.sync.dma_start(out=bp_sb, in_=bp_d)
        ones1 = consts.tile([1, 128], BF16, tag="ones1")
        nc.gpsimd.memset(ones1, 1.0)
        eps_t = consts.tile([128, 1], F32, tag="eps")
        nc.vector.memset(eps_t, LN_EPS)
        ident = consts.tile([128, 128], BF16, tag="ident")
        make_identity(nc, ident)

        # ---- persistent activations ----
        # vw holds VSCALE*v (+ VSCALE ones column) in fp8 for DoubleRow AV
        vw_sb = acts.tile([128, KT, H, HD + 1], F8, tag="vw")  # [kv-part, kv-tile, h, d+1]
        attn_sb = acts.tile([128, NT, H, HD], BF16, tag="attn")  # [n-part, n-tile, h, d]
        v0row = consts.tile([1, DIM], BF16, tag="v0row")  # v[kv=0] kept bf16-precise

        # input loads, c-chunk granular; only xcatT is loaded up front --
        # wv/wp loads are emitted later, in consumption order
        xcatT_sb = ph1.tile([128, CC, KVP], BF16, tag="xcatT")
        # wv and wp share one slot: wv dies after the v projection, wp is
        # only needed from the output projection onwards
        wv_sb = ph1.tile([128, CC, DIM], BF16, tag="wvwp")
        w0q = wstream.tile([128, CC, 128], BF16, tag="w")
        nc.sync.dma_start(out=w0q, in_=wq_re[:, :, 0:128])
        w0k = wstream.tile([128, CC, 128], BF16, tag="w")
        nc.sync.dma_start(out=w0k, in_=wk_re[:, :, 0:128])
        for cc in range(CC):
            dmae = nc.sync if cc % 3 != 2 else nc.gpsimd
            dmae.dma_start(out=xcatT_sb[:, cc, :], in_=xcat_re[:, cc, :])

        # ---- q/k projections interleaved with their dependent head pairs,
        # so ScalarE (exp) fills while PE still runs projections ----
        last_rows = KV - (KT - 1) * 128  # 77
        ksplits = [(0, 512), (512, 512), (1024, 128)]

        def emit_vproj_chunk(kvt, half):
            # one (kv-tile, head-half) of the v projection, natural [kv, o]
            ps = ps_proj.tile([128, 512], F32, tag="ps")
            for cc in range(CC):
                nc.tensor.matmul(
                    ps,
                    xcatT_sb[:, cc, kvt * 128:(kvt + 1) * 128],
                    wv_sb[:, cc, half * 512:(half + 1) * 512],
                    start=(cc == 0),
                    stop=(cc == CC - 1),
                )
            if kvt == 0:
                # bf16-precise first-key row for the gate term
                nc.vector.tensor_copy(
                    v0row[0:1, half * 512:(half + 1) * 512], ps[0:1, :])
            nc.vector.tensor_scalar_mul(
                vw_sb[:, kvt, half * 8:(half + 1) * 8, 0:HD],
                ps.rearrange("p (h d) -> p h d", d=HD),
                VSCALE,
            )

        # PE work other than the score matmuls is drip-fed between score
        # tiles via a budget-paced filler queue, so the PE never runs a
        # long burst that starves ScalarE of score psums. Entries are
        # (group, est_us, thunk); pop_fill spends ~a slot's budget, and
        # drain_group forces everything up to a group out (emission-order
        # dependencies: qk(p) must be emitted before scores(p) reads it).
        fill_q = []

        def pop_fill(budget=1.3):
            spent = 0.0
            while fill_q and spent < budget:
                _, c, t = fill_q.pop(0)
                t()
                spent += c

        def drain_group(g):
            # groups touch disjoint tiles, so popping out of FIFO order is
            # safe; only the named group is forced out
            rest = []
            for e in fill_q:
                if e[0] == g:
                    e[2]()
                else:
                    rest.append(e)
            fill_q[:] = rest

        def qk_q_half(qt, wtq, half):
            ps = ps_proj.tile([128, 512], F32, tag="ps")
            for cc in range(CC):
                nc.tensor.matmul(
                    ps,
                    wtq[:, cc, :],
                    xcatT_sb[:, cc, P + half * 512: P + (half + 1) * 512],
                    start=(cc == 0),
                    stop=(cc == CC - 1),
                )
            nc.vector.tensor_copy(qt[:, half * 512:(half + 1) * 512], ps)

        def qk_k_chunk(kte, kto, wtk, off, width):
            # k is split per head into K=128 zero-padded tiles (kte rows
            # 64:128 and kto rows 0:64 stay zero) so the score matmuls
            # contract a full 128 partitions (keeps fast-weight-load on)
            ps = ps_proj.tile([128, 512], F32, tag="ps")
            for cc in range(CC):
                nc.tensor.matmul(
                    ps[:, :width],
                    wtk[:, cc, :],
                    xcatT_sb[:, cc, off:off + width],
                    start=(cc == 0),
                    stop=(cc == CC - 1),
                )
            nc.vector.tensor_copy(kte[0:64, off:off + width], ps[0:64, :width])
            nc.vector.tensor_copy(kto[64:128, off:off + width], ps[64:128, :width])

        def push_qk(ot, wtq=None, wtk=None, direct=False):
            qt = qkp.tile([128, N], BF16, tag="qt")
            kte = qkp.tile([128, KVP], BF16, tag="kte")
            kto = qkp.tile([128, KVP], BF16, tag="kto")
            nc.gpsimd.memset(kte[64:128, :], 0.0)
            nc.gpsimd.memset(kto[0:64, :], 0.0)
            if wtq is None:
                wtq = wstream.tile([128, CC, 128], BF16, tag="w")
                nc.sync.dma_start(out=wtq, in_=wq_re[:, :, ot * 128:(ot + 1) * 128])
            if wtk is None:
                wtk = wstream.tile([128, CC, 128], BF16, tag="w")
                nc.sync.dma_start(out=wtk, in_=wk_re[:, :, ot * 128:(ot + 1) * 128])
            grp = f"qk{ot}"
            for half in range(2):
                fill_q.append((grp, 1.73,
                               lambda half=half: qk_q_half(qt, wtq, half)))
            for off, width in ksplits:
                fill_q.append((grp, 1.73 * width / 512,
                               lambda off=off, width=width:
                               qk_k_chunk(kte, kto, wtk, off, width)))
            if direct:
                drain_group(grp)
            return qt, (kte, kto)

        def emit_scores_pair(qt, kt):
            # Scores for the even/odd head pair, row-tiled on the PE
            # (K=64 each, partitions 0-63 and 64-127 run concurrently).
            ee = epool.tile([128, KT, N], F8, tag="e")
            eo = epool.tile([128, KT, N], F8, tag="e")
            nc.gpsimd.memset(ee[:, KT - 1, :], 0.0)
            nc.gpsimd.memset(eo[:, KT - 1, :], 0.0)
            kte, kto = kt
            for kvt in range(KT):
                pse = ps_scores.tile([128, N], F32, tag="pss")
                pso = ps_scores.tile([128, N], F32, tag="pss")
                for half in range(2):
                    nc.tensor.matmul(
                        pse[:, half * 512:(half + 1) * 512],
                        kte[:, kvt * 128:(kvt + 1) * 128],
                        qt[:, half * 512:(half + 1) * 512],
                        start=True, stop=True,
                    )
                    nc.tensor.matmul(
                        pso[:, half * 512:(half + 1) * 512],
                        kto[:, kvt * 128:(kvt + 1) * 128],
                        qt[:, half * 512:(half + 1) * 512],
                        start=True, stop=True,
                    )
                rows = last_rows if kvt == KT - 1 else 128
                nc.scalar.activation(
                    ee[:rows, kvt, :], pse[:rows], AF.Exp, bias=0.0, scale=0.125)
                nc.scalar.activation(
                    eo[:rows, kvt, :], pso[:rows], AF.Exp, bias=0.0, scale=0.125)
                pop_fill()
            # first key column is gated separately
            nc.gpsimd.memset(ee[0:1, 0, :], 0.0)
            nc.gpsimd.memset(eo[0:1, 0, :], 0.0)
            return ee, eo

        # fp8 AV: plain (non-DoubleRow) keeps fast-weight-load enabled,
        # which wins for this small free dim (65) where LDWEIGHTS dominates
        AV_DR = bool(int(os.environ.get("BASS_AV_DR", "0")))

        def av_chunk(h, e, nts, gv0s, alt):
            for nt in nts:
                if alt and nt % 2 == 1:
                    avp = ps_proj.tile([128, HD + 1], F32, tag="ps")
                else:
                    avp = ps_av.tile([128, HD + 1], F32, tag="avp")
                if AV_DR:
                    # DoubleRow: contract two kv-tiles per matmul
                    for j in range(KT // 2):
                        nc.tensor.matmul(
                            avp,
                            e[:, 2 * j:2 * j + 2, nt * 128:(nt + 1) * 128],
                            vw_sb[:, 2 * j:2 * j + 2, h, :],
                            start=(j == 0),
                            stop=False,
                            perf_mode=DR,
                        )
                else:
                    for j in range(KT - 1):
                        nc.tensor.matmul(
                            avp,
                            e[:, j, nt * 128:(nt + 1) * 128],
                            vw_sb[:, j, h, :],
                            start=(j == 0),
                            stop=False,
                        )
                nc.tensor.matmul(
                    avp,
                    e[:, KT - 1, nt * 128:(nt + 1) * 128],
                    vw_sb[:, KT - 1, h, :],
                    start=False,
                    stop=True,
                )
                rs = tpool.tile([128, 1], F32, tag="rs")
                nc.vector.reciprocal(rs, avp[:, HD:HD + 1])
                nc.vector.scalar_tensor_tensor(
                    out=attn_sb[:, nt, h, :],
                    in0=avp[:, 0:HD],
                    scalar=rs,
                    in1=gv0s,
                    op0=OP.mult,
                    op1=OP.add,
                )

        def push_av(pair, es, alt=False, nt_major=False):
            """Queue the AV + fixup work for both heads of a pair as small
            fillers. nt_major orders chunks so attn n-tiles complete in
            order (lets the LN/out-proj pipeline start during the drain)."""
            gv = []
            for i, h in enumerate((2 * pair, 2 * pair + 1)):
                gv0 = tpool.tile([128, HD], BF16, tag="gv0")
                nc.gpsimd.partition_broadcast(
                    gv0, v0row[0:1, h * HD:(h + 1) * HD])
                gv0s = tpool.tile([128, HD], F32, tag="gv0s")
                nc.vector.tensor_scalar_mul(gv0s, gv0, tanhg_sb[:, h:h + 1])
                gv.append(gv0s)
            grp = f"av{pair}"
            for nt0 in range(0, NT, 2):
                for i, h in enumerate((2 * pair, 2 * pair + 1)):
                    nts = [nt0, nt0 + 1]
                    fill_q.append(
                        (grp, 0.6,
                         lambda h=h, e=es[i], nts=tuple(nts), g=gv[i]:
                         av_chunk(h, e, nts, g, alt)))

        # ---- software pipeline over head pairs: scores+exp stream on
        # PE+ScalarE while qk projections, the v projection, and the AV
        # stages of earlier pairs fill the PE between score tiles ----
        pend = []
        qt, kt = push_qk(0, w0q, w0k, direct=True)
        for cc in range(CC):
            nc.sync.dma_start(out=wv_sb[:, cc, :], in_=wv_re[:, cc, :])
        # ones column for the row-sum S (E rows for kv=0/pad are zeroed);
        # disjoint from the v-projection's columns, so set it up front
        nc.gpsimd.memset(vw_sb[:, :, :, HD:HD + 1], VSCALE)
        nxt = push_qk(1)
        for kvt in range(KT):
            for half in range(2):
                fill_q.append(("vproj", 1.73,
                               lambda kvt=kvt, half=half:
                               emit_vproj_chunk(kvt, half)))
        pend.append(emit_scores_pair(qt, kt))
        for p in range(1, OT):
            qt, kt = nxt
            if p + 1 < OT:
                nxt = push_qk(p + 1)
            if p >= 3:
                push_av(p - 3, pend[p - 3], alt=False)
            drain_group(f"qk{p}")
            pend.append(emit_scores_pair(qt, kt))
        wp_sb = ph1.tile([128, CC, DIM], BF16, tag="wvwp")
        for cc in range(CC):
            nc.sync.dma_start(out=wp_sb[:, cc, :], in_=wp_re[:, cc, :])
        # pairs 5..7 drain after the last scores; keep nt-major order on the
        # final pair so the LN/out-proj pipeline can start as tiles finish
        push_av(5, pend[5], alt=True)
        push_av(6, pend[6], alt=True)
        push_av(7, pend[7], alt=True, nt_major=True)
        while fill_q:
            _, _, t = fill_q.pop(0)
            t()

        # ---- LayerNorm + output projection per n-tile; LN emitted one
        # n-tile ahead so the LN chain of nt+1 overlaps the projection of nt ----
        def emit_ln(nt):
            xa = attn_sb[:, nt].rearrange("p h d -> p (h d)")
            xs = xa.rearrange("p (s f) -> p s f", f=512)
            stats = tpool.tile([128, 2, 6], F32, tag="stats")
            for s in range(2):
                nc.vector.bn_stats(stats[:, s, :], xs[:, s, :])
            mv = tpool.tile([128, 2], F32, tag="mv")
            nc.vector.bn_aggr(mv, stats)
            rstd = tpool.tile([128, 1], F32, tag="rstd")
            nc.scalar.activation(rstd, mv[:, 1:2], AF.Sqrt, bias=eps_t, scale=1.0)
            nc.vector.reciprocal(rstd, rstd)
            # ln_g/ln_b are folded into Wp/bp host-side: L = (x - mu) * rstd
            L_t = qkp.tile([128, DIM], BF16, tag="qt")
            nc.vector.tensor_scalar(
                out=L_t, in0=xa, scalar1=mv[:, 0:1], scalar2=rstd,
                op0=OP.subtract, op1=OP.mult,
            )
            return L_t

        L_t = emit_ln(0)
        for nt in range(NT):
            L_next = emit_ln(nt + 1) if nt + 1 < NT else None

            # transpose LN rows then project: out[n, o] = L @ Wp'.T + bp'
            pp0 = ps_proj.tile([128, 512], F32, tag="ps")
            pp1 = ps_proj.tile([128, 512], F32, tag="ps")
            for cc in range(CC):
                pstp, pstt = (ps_scores, "pss") if cc % 2 == 0 else (ps_av, "avp")
                pst = pstp.tile([128, 128], BF16, tag=pstt)
                nc.tensor.transpose(
                    pst, L_t[:, cc * 128:(cc + 1) * 128], ident
                )
                ltc = ltp.tile([128, 128], BF16, tag="ltc")
                nc.vector.tensor_copy(ltc, pst)
                nc.tensor.matmul(
                    pp0, ltc, wp_sb[:, cc, 0:512],
                    start=(cc == 0), stop=False,
                )
                nc.tensor.matmul(
                    pp1, ltc, wp_sb[:, cc, 512:1024],
                    start=(cc == 0), stop=False,
                )
            # bias as rank-1 accumulation (PSUM is not a legal DMA source,
            # so stage through SBUF)
            nc.tensor.matmul(pp0, ones1, bp_sb[:, 0:512], start=False, stop=True)
            nc.tensor.matmul(pp1, ones1, bp_sb[:, 512:1024], start=False, stop=True)
            ot0 = opool.tile([128, 512], F32, tag="ot")
            nc.scalar.copy(out=ot0, in_=pp0)
            nc.sync.dma_start(out=out_d[nt * 128:(nt + 1) * 128, 0:512], in_=ot0)
            ot1 = opool.tile([128, 512], F32, tag="ot")
            nc.scalar.copy(out=ot1, in_=pp1)
            nc.sync.dma_start(out=out_d[nt * 128:(nt + 1) * 128, 512:1024], in_=ot1)
            L_t = L_next


def build_program():
    if "nc" in _CACHE:
        return _CACHE["nc"]
    nc = bacc.Bacc("TRN2", target_bir_lowering=False, debug=False, num_devices=8)
    with tile.TileContext(nc) as tc:
        _emit(tc)
    nc.compile()
    _CACHE["nc"] = nc
    return nc


def prep_inputs(x, x_text, Wq, Wk, Wv, gate, ln_g, ln_b, Wp, bp):
    """Host-side sharding/layout prep. Returns the 8 per-core input maps."""
    bf = ml_dtypes.bfloat16
    x = np.asarray(x, np.float32)
    x_text = np.asarray(x_text, np.float32)
    xcat = np.concatenate([x_text, x], axis=1)          # [B, KV, DIM]
    xcatT = np.zeros((B, DIM, KVP), np.float32)
    xcatT[:, :, :KV] = xcat.transpose(0, 2, 1)
    xcatT = xcatT.astype(bf)
    wqT = np.ascontiguousarray(np.asarray(Wq, np.float32).T).astype(bf)
    wkT = np.ascontiguousarray(np.asarray(Wk, np.float32).T).astype(bf)
    wvT = np.ascontiguousarray(np.asarray(Wv, np.float32).T).astype(bf)
    # fold LayerNorm affine into the output projection:
    #   (L*g + b) @ Wp.T + bp == L @ (Wp*g).T + (bp + Wp @ b)
    Wp = np.asarray(Wp, np.float32)
    g = np.asarray(ln_g, np.float32).reshape(DIM)
    bvec = np.asarray(ln_b, np.float32).reshape(DIM)
    Wpf = Wp * g[None, :]
    bpf = np.asarray(bp, np.float32).reshape(DIM) + Wp @ bvec
    wpT = np.ascontiguousarray(Wpf.T).astype(bf)
    tanhg = np.tanh(np.asarray(gate, np.float32)).reshape(1, H).astype(np.float32)
    bp_bf = bpf.reshape(1, DIM).astype(bf)
    in_maps = []
    for b in range(B):
        in_maps.append({
            "xcatT": np.ascontiguousarray(xcatT[b]),
            "wqT": wqT, "wkT": wkT, "wvT": wvT, "wpT": wpT,
            "tanhg": tanhg, "bp_bf": bp_bf,
        })
    return in_maps


def kernel(**inputs):
    global LAST_EXEC_NS
    nc = build_program()
    in_maps = prep_inputs(**inputs)
    trace = bool(int(os.environ.get("BASS_TRACE_RUN", "0")))
    res = run_bass_kernel_spmd(
        nc, in_maps, core_ids=list(range(8)), trace=trace,
    )
    LAST_EXEC_NS = res.exec_time_ns
    out = np.stack([r["out"] for r in res.results], axis=0)
    return out.astype(np.float32)



# revision 45
# speedup vs baseline: 3.4483x; 3.2268x over previous
"""Trainium2 Bass kernel for nn_Attention_45724221833663 (sparse_attention).

Strategy: data-parallel over batch B=8 across the 8 NeuronCores (one batch
element per core). All matmuls run in bf16 with fp32 PSUM accumulation.

Per-core dataflow (all layouts chosen to avoid on-chip transposes of large
activations; weights and x are transposed on the host while sharding, and
ln_g/ln_b are folded into Wp/bp on the host):
  xcatT  [c=1024, kvp=1152]  (= concat(x_text, x).T, zero-padded 1101->1152)
  vw     [kvp, h, 65] = (xcatT.T @ WvT) interleaved per head + ones column
  qT     [o, n]    = WqT.T @ xT          (o = head-major channel)
  kT     [o, kvp]  = WkT.T @ xcatT
  per head h:
    scoresT[kv, n] = kT_h contracted with qT_h over d=64
    E = exp(scoresT / 8)     (ScalarE, psum -> sbuf bf16); row kv=0 and the
                             pad rows are zeroed
    avp[n,0:65] = sum_kv E[kv,n-tile] * vw[kv, h, :]   (col 64 = S[n])
    attn[n, h*64:+64] = avp[:, :64] * (1/S) + tanh(g_h) * v_h[kv=0]
  LayerNorm over channels (rows of attn, bf16 input like the reference's
  bf16 cast; ln_g/ln_b pre-folded), then out = LN @ Wp'.T + bp' with the
  bias added as a rank-1 matmul and the result DMA'd PSUM -> DRAM.
"""

import os
import numpy as np
import ml_dtypes

import concourse.bacc as bacc
import concourse.tile as tile
from concourse import mybir
from concourse.masks import make_identity
from concourse.bass_utils import run_bass_kernel_spmd

F32 = mybir.dt.float32
BF16 = mybir.dt.bfloat16
F8 = mybir.dt.float8e4
DR = mybir.MatmulPerfMode.DoubleRow
AF = mybir.ActivationFunctionType
OP = mybir.AluOpType
VSCALE = 16.0  # fp8 v/ones scaling; cancels in avp * (1/S-column)

B, N, P, DIM, H = 8, 1024, 77, 1024, 16
HD = DIM // H          # 64
KV = P + N             # 1101
KT = 9                 # kv tiles of 128
KVP = KT * 128         # 1152 padded
NT = N // 128          # 8 n tiles
CC = DIM // 128        # 8 contraction chunks
OT = DIM // 128        # 8 output-channel tiles
LN_EPS = 1e-5

LAST_EXEC_NS = None
_CACHE = {}


def _emit(tc):
    nc = tc.nc

    xcatT_d = nc.dram_tensor("xcatT", [DIM, KVP], BF16, kind="ExternalInput").ap()
    wq_d = nc.dram_tensor("wqT", [DIM, DIM], BF16, kind="ExternalInput").ap()
    wk_d = nc.dram_tensor("wkT", [DIM, DIM], BF16, kind="ExternalInput").ap()
    wv_d = nc.dram_tensor("wvT", [DIM, DIM], BF16, kind="ExternalInput").ap()
    wp_d = nc.dram_tensor("wpT", [DIM, DIM], BF16, kind="ExternalInput").ap()
    tanhg_d = nc.dram_tensor("tanhg", [1, H], F32, kind="ExternalInput").ap()
    bp_d = nc.dram_tensor("bp_bf", [1, DIM], BF16, kind="ExternalInput").ap()
    out_d = nc.dram_tensor("out", [N, DIM], F32, kind="ExternalOutput").ap()

    xcat_re = xcatT_d.rearrange("(j p) f -> p j f", p=128)
    wq_re = wq_d.rearrange("(j p) o -> p j o", p=128)
    wk_re = wk_d.rearrange("(j p) o -> p j o", p=128)
    wv_re = wv_d.rearrange("(j p) o -> p j o", p=128)
    wp_re = wp_d.rearrange("(j p) o -> p j o", p=128)

    from contextlib import ExitStack

    with ExitStack() as top:
        consts = top.enter_context(tc.tile_pool(name="consts", bufs=1))
        acts = top.enter_context(tc.tile_pool(name="acts", bufs=1))
        ph1 = top.enter_context(tc.tile_pool(name="ph1", bufs=1))
        wstream = top.enter_context(tc.tile_pool(name="wstream", bufs=3))
        qkp = top.enter_context(tc.tile_pool(name="qkp", bufs=7))
        epool = top.enter_context(tc.tile_pool(name="epool", bufs=9))
        tpool = top.enter_context(tc.tile_pool(name="tmp", bufs=4))
        ltp = top.enter_context(tc.tile_pool(name="ltp", bufs=6))
        opool = top.enter_context(tc.tile_pool(name="outp", bufs=3))
        ps_proj = top.enter_context(tc.tile_pool(name="ps_proj", bufs=2, space="PSUM"))
        ps_scores = top.enter_context(
            tc.tile_pool(name="ps_scores", bufs=2, space="PSUM"))
        ps_av = top.enter_context(tc.tile_pool(name="ps_av", bufs=2, space="PSUM"))

        # ---- constants ---- (tanhg/bp DMAs are emitted after the first
        # score pair: every dma_start costs ~0.8us of DIRECT2D trigger time
        # on the issuing sequencer, and these must not delay the xcat load)
        tanhg_sb = consts.tile([128, H], F32, tag="tanhg")
        bp_sb = consts.tile([1, DIM], BF16, tag="bp")
        ones1 = consts.tile([1, 128], BF16, tag="ones1")
        nc.gpsimd.memset(ones1, 1.0)
        eps_t = consts.tile([128, 1], F32, tag="eps")
        nc.vector.memset(eps_t, LN_EPS)
        ident = consts.tile([128, 128], BF16, tag="ident")
        make_identity(nc, ident)

        # ---- persistent activations ----
        # vw holds VSCALE*v (+ VSCALE ones column) in fp8 for DoubleRow AV
        vw_sb = acts.tile([128, KT, H, HD + 1], F8, tag="vw")  # [kv-part, kv-tile, h, d+1]
        attn_sb = acts.tile([128, NT, H, HD], BF16, tag="attn")  # [n-part, n-tile, h, d]
        v0row = consts.tile([1, DIM], BF16, tag="v0row")  # v[kv=0] kept bf16-precise

        # input loads, c-chunk granular; only xcatT is loaded up front --
        # wv/wp loads are emitted later, in consumption order
        xcatT_sb = ph1.tile([128, CC, KVP], BF16, tag="xcatT")
        # wv and wp share one slot: wv dies after the v projection, wp is
        # only needed from the output projection onwards
        wv_sb = ph1.tile([128, CC, DIM], BF16, tag="wvwp")
        # first-pair weights on the scalar queue (off the critical sync
        # queue); xcat split into two fused DMAs so only 2 triggers stand
        # between program start and the first q-projection matmuls
        w0q = wstream.tile([128, CC, 128], BF16, tag="w")
        nc.scalar.dma_start(out=w0q, in_=wq_re[:, :, 0:128])
        w0k = wstream.tile([128, CC, 128], BF16, tag="w")
        nc.scalar.dma_start(out=w0k, in_=wk_re[:, :, 0:128])
        nc.sync.dma_start(out=xcatT_sb[:, 0:3, :], in_=xcat_re[:, 0:3, :])
        nc.gpsimd.dma_start(out=xcatT_sb[:, 3:CC, :], in_=xcat_re[:, 3:CC, :])

        # ---- q/k projections interleaved with their dependent head pairs,
        # so ScalarE (exp) fills while PE still runs projections ----
        last_rows = KV - (KT - 1) * 128  # 77
        ksplits = [(0, 512), (512, 512), (1024, 128)]

        def emit_vproj_chunk(kvt, half):
            # one (kv-tile, head-half) of the v projection, natural [kv, o]
            ps = ps_proj.tile([128, 512], F32, tag="ps")
            for cc in range(CC):
                nc.tensor.matmul(
                    ps,
                    xcatT_sb[:, cc, kvt * 128:(kvt + 1) * 128],
                    wv_sb[:, cc, half * 512:(half + 1) * 512],
                    start=(cc == 0),
                    stop=(cc == CC - 1),
                )
            if kvt == 0:
                # bf16-precise first-key row for the gate term
                nc.vector.tensor_copy(
                    v0row[0:1, half * 512:(half + 1) * 512], ps[0:1, :])
            nc.vector.tensor_scalar_mul(
                vw_sb[:, kvt, half * 8:(half + 1) * 8, 0:HD],
                ps.rearrange("p (h d) -> p h d", d=HD),
                VSCALE,
            )

        # PE work other than the score matmuls is drip-fed between score
        # tiles via a budget-paced filler queue, so the PE never runs a
        # long burst that starves ScalarE of score psums. Entries are
        # (group, est_us, thunk); pop_fill spends ~a slot's budget, and
        # drain_group forces everything up to a group out (emission-order
        # dependencies: qk(p) must be emitted before scores(p) reads it).
        fill_q = []

        def pop_fill(budget=1.3):
            spent = 0.0
            while fill_q and spent < budget:
                _, c, t = fill_q.pop(0)
                t()
                spent += c

        def drain_group(g):
            # groups touch disjoint tiles, so popping out of FIFO order is
            # safe; only the named group is forced out
            rest = []
            for e in fill_q:
                if e[0] == g:
                    e[2]()
                else:
                    rest.append(e)
            fill_q[:] = rest

        def qk_q_half(qt, wtq, half):
            ps = ps_proj.tile([128, 512], F32, tag="ps")
            for cc in range(CC):
                nc.tensor.matmul(
                    ps,
                    wtq[:, cc, :],
                    xcatT_sb[:, cc, P + half * 512: P + (half + 1) * 512],
                    start=(cc == 0),
                    stop=(cc == CC - 1),
                )
            nc.vector.tensor_copy(qt[:, half * 512:(half + 1) * 512], ps)

        def qk_k_chunk(kte, kto, wtk, off, width):
            # k is split per head into K=128 zero-padded tiles (kte rows
            # 64:128 and kto rows 0:64 stay zero) so the score matmuls
            # contract a full 128 partitions (keeps fast-weight-load on)
            ps = ps_proj.tile([128, 512], F32, tag="ps")
            for cc in range(CC):
                nc.tensor.matmul(
                    ps[:, :width],
                    wtk[:, cc, :],
                    xcatT_sb[:, cc, off:off + width],
                    start=(cc == 0),
                    stop=(cc == CC - 1),
                )
            nc.vector.tensor_copy(kte[0:64, off:off + width], ps[0:64, :width])
            nc.vector.tensor_copy(kto[64:128, off:off + width], ps[64:128, :width])

        def push_qk(ot, wtq=None, wtk=None, direct=False):
            qt = qkp.tile([128, N], BF16, tag="qt")
            kte = qkp.tile([128, KVP], BF16, tag="kte")
            kto = qkp.tile([128, KVP], BF16, tag="kto")
            nc.gpsimd.memset(kte[64:128, :], 0.0)
            nc.gpsimd.memset(kto[0:64, :], 0.0)
            if wtq is None:
                wtq = wstream.tile([128, CC, 128], BF16, tag="w")
                nc.sync.dma_start(out=wtq, in_=wq_re[:, :, ot * 128:(ot + 1) * 128])
            if wtk is None:
                wtk = wstream.tile([128, CC, 128], BF16, tag="w")
                nc.sync.dma_start(out=wtk, in_=wk_re[:, :, ot * 128:(ot + 1) * 128])
            grp = f"qk{ot}"
            for half in range(2):
                fill_q.append((grp, 1.73,
                               lambda half=half: qk_q_half(qt, wtq, half)))
            for off, width in ksplits:
                fill_q.append((grp, 1.73 * width / 512,
                               lambda off=off, width=width:
                               qk_k_chunk(kte, kto, wtk, off, width)))
            if direct:
                drain_group(grp)
            return qt, (kte, kto)

        def emit_scores_pair(qt, kt):
            # Scores for the even/odd head pair, row-tiled on the PE
            # (K=64 each, partitions 0-63 and 64-127 run concurrently).
            ee = epool.tile([128, KT, N], F8, tag="e")
            eo = epool.tile([128, KT, N], F8, tag="e")
            nc.gpsimd.memset(ee[:, KT - 1, :], 0.0)
            nc.gpsimd.memset(eo[:, KT - 1, :], 0.0)
            kte, kto = kt
            for kvt in range(KT):
                pse = ps_scores.tile([128, N], F32, tag="pss")
                pso = ps_scores.tile([128, N], F32, tag="pss")
                for half in range(2):
                    nc.tensor.matmul(
                        pse[:, half * 512:(half + 1) * 512],
                        kte[:, kvt * 128:(kvt + 1) * 128],
                        qt[:, half * 512:(half + 1) * 512],
                        start=True, stop=True,
                    )
                    nc.tensor.matmul(
                        pso[:, half * 512:(half + 1) * 512],
                        kto[:, kvt * 128:(kvt + 1) * 128],
                        qt[:, half * 512:(half + 1) * 512],
                        start=True, stop=True,
                    )
                rows = last_rows if kvt == KT - 1 else 128
                nc.scalar.activation(
                    ee[:rows, kvt, :], pse[:rows], AF.Exp, bias=0.0, scale=0.125)
                nc.scalar.activation(
                    eo[:rows, kvt, :], pso[:rows], AF.Exp, bias=0.0, scale=0.125)
                pop_fill()
            # first key column is gated separately
            nc.gpsimd.memset(ee[0:1, 0, :], 0.0)
            nc.gpsimd.memset(eo[0:1, 0, :], 0.0)
            return ee, eo

        # fp8 AV: plain (non-DoubleRow) keeps fast-weight-load enabled,
        # which wins for this small free dim (65) where LDWEIGHTS dominates
        AV_DR = bool(int(os.environ.get("BASS_AV_DR", "0")))

        def av_chunk(h, e, nts, gv0s, alt):
            for nt in nts:
                if alt and nt % 2 == 1:
                    avp = ps_proj.tile([128, HD + 1], F32, tag="ps")
                else:
                    avp = ps_av.tile([128, HD + 1], F32, tag="avp")
                if AV_DR:
                    # DoubleRow: contract two kv-tiles per matmul
                    for j in range(KT // 2):
                        nc.tensor.matmul(
                            avp,
                            e[:, 2 * j:2 * j + 2, nt * 128:(nt + 1) * 128],
                            vw_sb[:, 2 * j:2 * j + 2, h, :],
                            start=(j == 0),
                            stop=False,
                            perf_mode=DR,
                        )
                else:
                    for j in range(KT - 1):
                        nc.tensor.matmul(
                            avp,
                            e[:, j, nt * 128:(nt + 1) * 128],
                            vw_sb[:, j, h, :],
                            start=(j == 0),
                            stop=False,
                        )
                nc.tensor.matmul(
                    avp,
                    e[:, KT - 1, nt * 128:(nt + 1) * 128],
                    vw_sb[:, KT - 1, h, :],
                    start=False,
                    stop=True,
                )
                rs = tpool.tile([128, 1], F32, tag="rs")
                nc.vector.reciprocal(rs, avp[:, HD:HD + 1])
                nc.vector.scalar_tensor_tensor(
                    out=attn_sb[:, nt, h, :],
                    in0=avp[:, 0:HD],
                    scalar=rs,
                    in1=gv0s,
                    op0=OP.mult,
                    op1=OP.add,
                )

        def push_av(pair, es, alt=False, nt_major=False):
            """Queue the AV + fixup work for both heads of a pair as small
            fillers. nt_major orders chunks so attn n-tiles complete in
            order (lets the LN/out-proj pipeline start during the drain)."""
            gv = []
            for i, h in enumerate((2 * pair, 2 * pair + 1)):
                gv0 = tpool.tile([128, HD], BF16, tag="gv0")
                nc.gpsimd.partition_broadcast(
                    gv0, v0row[0:1, h * HD:(h + 1) * HD])
                gv0s = tpool.tile([128, HD], F32, tag="gv0s")
                nc.vector.tensor_scalar_mul(gv0s, gv0, tanhg_sb[:, h:h + 1])
                gv.append(gv0s)
            grp = f"av{pair}"
            for nt0 in range(0, NT, 2):
                for i, h in enumerate((2 * pair, 2 * pair + 1)):
                    nts = [nt0, nt0 + 1]
                    fill_q.append(
                        (grp, 0.6,
                         lambda h=h, e=es[i], nts=tuple(nts), g=gv[i]:
                         av_chunk(h, e, nts, g, alt)))

        # ---- software pipeline over head pairs: scores+exp stream on
        # PE+ScalarE while qk projections, the v projection, and the AV
        # stages of earlier pairs fill the PE between score tiles ----
        pend = []
        qt, kt = push_qk(0, w0q, w0k, direct=True)
        nc.sync.dma_start(out=wv_sb, in_=wv_re)
        # ones column for the row-sum S (E rows for kv=0/pad are zeroed);
        # disjoint from the v-projection's columns, so set it up front
        nc.gpsimd.memset(vw_sb[:, :, :, HD:HD + 1], VSCALE)
        nxt = push_qk(1)
        for kvt in range(KT):
            for half in range(2):
                fill_q.append(("vproj", 1.73,
                               lambda kvt=kvt, half=half:
                               emit_vproj_chunk(kvt, half)))
        pend.append(emit_scores_pair(qt, kt))
        # deferred constant loads (needed from the AV stage onwards)
        nc.sync.dma_start(out=tanhg_sb, in_=tanhg_d.to_broadcast([128, H]))
        nc.sync.dma_start(out=bp_sb, in_=bp_d)
        for p in range(1, OT):
            qt, kt = nxt
            if p + 1 < OT:
                nxt = push_qk(p + 1)
            if p >= 3:
                push_av(p - 3, pend[p - 3], alt=False)
            drain_group(f"qk{p}")
            pend.append(emit_scores_pair(qt, kt))
        wp_sb = ph1.tile([128, CC, DIM], BF16, tag="wvwp")
        nc.sync.dma_start(out=wp_sb, in_=wp_re)
        # pairs 5..7 drain after the last scores; keep nt-major order on the
        # final pair so the LN/out-proj pipeline can start as tiles finish
        push_av(5, pend[5], alt=True)
        push_av(6, pend[6], alt=True)
        push_av(7, pend[7], alt=True, nt_major=True)
        while fill_q:
            _, _, t = fill_q.pop(0)
            t()

        # ---- LayerNorm + output projection per n-tile; LN emitted one
        # n-tile ahead so the LN chain of nt+1 overlaps the projection of nt ----
        def emit_ln(nt):
            xa = attn_sb[:, nt].rearrange("p h d -> p (h d)")
            xs = xa.rearrange("p (s f) -> p s f", f=512)
            stats = tpool.tile([128, 2, 6], F32, tag="stats")
            for s in range(2):
                nc.vector.bn_stats(stats[:, s, :], xs[:, s, :])
            mv = tpool.tile([128, 2], F32, tag="mv")
            nc.vector.bn_aggr(mv, stats)
            rstd = tpool.tile([128, 1], F32, tag="rstd")
            nc.scalar.activation(rstd, mv[:, 1:2], AF.Sqrt, bias=eps_t, scale=1.0)
            nc.vector.reciprocal(rstd, rstd)
            # ln_g/ln_b are folded into Wp/bp host-side: L = (x - mu) * rstd
            L_t = qkp.tile([128, DIM], BF16, tag="qt")
            nc.vector.tensor_scalar(
                out=L_t, in0=xa, scalar1=mv[:, 0:1], scalar2=rstd,
                op0=OP.subtract, op1=OP.mult,
            )
            return L_t

        L_t = emit_ln(0)
        for nt in range(NT):
            L_next = emit_ln(nt + 1) if nt + 1 < NT else None

            # transpose LN rows then project: out[n, o] = L @ Wp'.T + bp'
            pp0 = ps_proj.tile([128, 512], F32, tag="ps")
            pp1 = ps_proj.tile([128, 512], F32, tag="ps")
            for cc in range(CC):
                pstp, pstt = (ps_scores, "pss") if cc % 2 == 0 else (ps_av, "avp")
                pst = pstp.tile([128, 128], BF16, tag=pstt)
                nc.tensor.transpose(
                    pst, L_t[:, cc * 128:(cc + 1) * 128], ident
                )
                ltc = ltp.tile([128, 128], BF16, tag="ltc")
                nc.vector.tensor_copy(ltc, pst)
                nc.tensor.matmul(
                    pp0, ltc, wp_sb[:, cc, 0:512],
                    start=(cc == 0), stop=False,
                )
                nc.tensor.matmul(
                    pp1, ltc, wp_sb[:, cc, 512:1024],
                    start=(cc == 0), stop=False,
                )
            # bias as rank-1 accumulation (PSUM is not a legal DMA source,
            # so stage through SBUF)
            nc.tensor.matmul(pp0, ones1, bp_sb[:, 0:512], start=False, stop=True)
            nc.tensor.matmul(pp1, ones1, bp_sb[:, 512:1024], start=False, stop=True)
            ot0 = opool.tile([128, 512], F32, tag="ot")
            nc.scalar.copy(out=ot0, in_=pp0)
            nc.sync.dma_start(out=out_d[nt * 128:(nt + 1) * 128, 0:512], in_=ot0)
            ot1 = opool.tile([128, 512], F32, tag="ot")
            nc.scalar.copy(out=ot1, in_=pp1)
            nc.sync.dma_start(out=out_d[nt * 128:(nt + 1) * 128, 512:1024], in_=ot1)
            L_t = L_next


def build_program():
    if "nc" in _CACHE:
        return _CACHE["nc"]
    nc = bacc.Bacc("TRN2", target_bir_lowering=False, debug=False, num_devices=8)
    with tile.TileContext(nc) as tc:
        _emit(tc)
    nc.compile()
    _CACHE["nc"] = nc
    return nc


def prep_inputs(x, x_text, Wq, Wk, Wv, gate, ln_g, ln_b, Wp, bp):
    """Host-side sharding/layout prep. Returns the 8 per-core input maps."""
    bf = ml_dtypes.bfloat16
    x = np.asarray(x, np.float32)
    x_text = np.asarray(x_text, np.float32)
    xcat = np.concatenate([x_text, x], axis=1)          # [B, KV, DIM]
    xcatT = np.zeros((B, DIM, KVP), np.float32)
    xcatT[:, :, :KV] = xcat.transpose(0, 2, 1)
    xcatT = xcatT.astype(bf)
    wqT = np.ascontiguousarray(np.asarray(Wq, np.float32).T).astype(bf)
    wkT = np.ascontiguousarray(np.asarray(Wk, np.float32).T).astype(bf)
    wvT = np.ascontiguousarray(np.asarray(Wv, np.float32).T).astype(bf)
    # fold LayerNorm affine into the output projection:
    #   (L*g + b) @ Wp.T + bp == L @ (Wp*g).T + (bp + Wp @ b)
    Wp = np.asarray(Wp, np.float32)
    g = np.asarray(ln_g, np.float32).reshape(DIM)
    bvec = np.asarray(ln_b, np.float32).reshape(DIM)
    Wpf = Wp * g[None, :]
    bpf = np.asarray(bp, np.float32).reshape(DIM) + Wp @ bvec
    wpT = np.ascontiguousarray(Wpf.T).astype(bf)
    tanhg = np.tanh(np.asarray(gate, np.float32)).reshape(1, H).astype(np.float32)
    bp_bf = bpf.reshape(1, DIM).astype(bf)
    in_maps = []
    for b in range(B):
        in_maps.append({
            "xcatT": np.ascontiguousarray(xcatT[b]),
            "wqT": wqT, "wkT": wkT, "wvT": wvT, "wpT": wpT,
            "tanhg": tanhg, "bp_bf": bp_bf,
        })
    return in_maps


def kernel(**inputs):
    global LAST_EXEC_NS
    nc = build_program()
    in_maps = prep_inputs(**inputs)
    trace = bool(int(os.environ.get("BASS_TRACE_RUN", "0")))
    res = run_bass_kernel_spmd(
        nc, in_maps, core_ids=list(range(8)), trace=trace,
    )
    LAST_EXEC_NS = res.exec_time_ns
    out = np.stack([r["out"] for r in res.results], axis=0)
    return out.astype(np.float32)

